# revision 1
# baseline (speedup 1.0000x reference)
"""Encoder-decoder attention (d_model=512, h=8 heads, d_k=d_v=64, S=2048),
head-parallel across 8 NeuronCores — one head per core.

Per-core Bass/Tile kernel (all layouts chosen so no on-chip transposes are
needed; host passes emb^T / K^T / V^T):
  qT[64,2048]  = (Wq_h^T emb^T) + bq  (bias folded in as a K=1 rank-update
                                       against an on-chip ones row)
  kT[64,2048]  = (Wk_h^T K^T)   + bk
  v[2048,65]   = (V Wv_h) + bv, with a 65th all-ones column
  ST[t,s]      = kT^T-tile @ qT  (scores transposed; K=64 contraction)
  E = exp(ST/8)                  (ScalarE, scale fused; no max-subtraction —
                                  |scores| <= ~2 for this problem's scale)
  psum_o[65,s] = [v|1]^T @ E     (accumulated over t; row 64 = softmax denom)
  partial      = (num^T @ Wo_h_rows) * (1/denom)  (divide folded past Wo as a
                                                   per-partition scale)
Host: out = sum over cores of partial + bo  (the unshard for row-sharded Wo).
"""

import numpy as np

import concourse.bass as bass
import concourse.mybir as mybir
import concourse.tile as tile
from concourse.bass_utils import run_bass_kernel_spmd

F32 = mybir.dt.float32
# "f32r" streams fp32 matmul operands over 4 XBUSes (1 cycle/row when the
# moving dim is >=256, vs 4 cycles/row for plain fp32) with identical
# numerics; flip to "f32" if hardware disagrees with the reference.
MM_MODE = "f32r"
D_MODEL, H, DK = 512, 8, 64
S = 2048  # both S_q and S_kv
NT = S // 128  # 16 key tiles
NSC = S // 512  # 4 query chunks
N_CORES = 8

# The walrus build in this container rejects >1 sync-wait per instruction.
# Tile freely attaches several waits to one instruction (multi-producer
# deps, the kernel-tail drain), so after scheduling, move all but the last
# wait of each instruction onto same-engine NoOps inserted just before it —
# the sequencer blocks on each in turn, which is semantically identical.
def _split_multi_waits(nc):
    n_split = 0
    for fn in nc.m.functions:
        for bb in fn.blocks:
            out = []
            for inst in bb.instructions:
                si = inst.sync_info
                waits = list(si.on_wait) if (si is not None and si.on_wait) else []
                if len(waits) > 1:
                    for w in waits[:-1]:
                        n_split += 1
                        nop = mybir.InstNoOp(
                            name=f"I-wsplit-{n_split}", ins=[], outs=[]
                        )
                        nop.engine = inst.engine
                        nop.sync_info = mybir.SyncInfo(on_wait=[w], on_update=[])
                        nc.register_instruction(nop, overwrite=True)
                        out.append(nop)
                    si.on_wait = [waits[-1]]
                out.append(inst)
            bb.instructions = out


def build_program(reps=1, ablate=()):
    """Build the per-core Bass program (same program on all 8 cores).
    reps>1 repeats the compute body in-NEFF (for device-time measurement:
    the R8-vs-R1 wall-clock difference cancels dispatch overhead).
    ablate: timing-only experiment switches ("exp", "scores", "attn",
    "proj", "epi") that drop pieces of the pipeline."""
    nc = bass.Bass("TRN2", target_bir_lowering=False, debug=False)

    # MDT: dtype of every SBUF tile that feeds a matmul. float32r streams
    # fp32 operands 4x faster through the PE; the BIR verifier requires the
    # producers (DMA loads, DVE/ACT evictions) to emit f32r themselves.
    MDT = mybir.dt.float32r if MM_MODE == "f32r" else F32

    def dma_in(dst_ap, src_ap):
        # DMA is a bit copy; bitcast the DRAM side to match f32r tiles.
        if MM_MODE == "f32r":
            src_ap = src_ap.bitcast(mybir.dt.float32r)
        nc.sync.dma_start(dst_ap, src_ap)

    embT = nc.dram_tensor("embT", [D_MODEL, S], F32, kind="ExternalInput").ap()
    kT_in = nc.dram_tensor("kT_in", [D_MODEL, S], F32, kind="ExternalInput").ap()
    vT_in = nc.dram_tensor("vT_in", [D_MODEL, S], F32, kind="ExternalInput").ap()
    wq_in = nc.dram_tensor("wq", [D_MODEL + 1, DK], F32, kind="ExternalInput").ap()
    wk_in = nc.dram_tensor("wk", [D_MODEL + 1, DK], F32, kind="ExternalInput").ap()
    wv_in = nc.dram_tensor("wv", [D_MODEL + 1, DK + 1], F32, kind="ExternalInput").ap()
    wo_in = nc.dram_tensor("wo", [DK, D_MODEL], F32, kind="ExternalInput").ap()
    out = nc.dram_tensor("out", [S, D_MODEL], F32, kind="ExternalOutput").ap()

    ND = D_MODEL // 128  # 4 contraction chunks

    with tile.TileContext(nc) as tc:
        with (
            tc.tile_pool(name="io", bufs=1) as iop,
            tc.tile_pool(name="wp", bufs=1) as wp,
            tc.tile_pool(name="cst", bufs=1) as cst,
            tc.tile_pool(name="qk", bufs=1) as qkp,
            tc.tile_pool(name="vp", bufs=1) as vp,
            tc.tile_pool(name="ep", bufs=10) as ep,
            tc.tile_pool(name="nump", bufs=3) as nump,
            tc.tile_pool(name="smp", bufs=6) as smp,
            tc.tile_pool(name="outp", bufs=4) as outp,
            tc.tile_pool(name="pacc", bufs=3, space="PSUM") as pacc,
            tc.tile_pool(name="po", bufs=2, space="PSUM") as pop,
            tc.tile_pool(name="pp", bufs=2, space="PSUM") as ppp,
            tc.tile_pool(name="pr", bufs=1, space="PSUM") as prp,
        ):
            # constants
            ones_row = cst.tile([1, S], F32, tag="ones_row")
            nc.vector.memset(ones_row[:], 1.0)
            ones_r = cst.tile([1, S], MDT, tag="ones_r")
            nc.vector.tensor_copy(ones_r[:], ones_row[:])
            one_one = cst.tile([1, 1], F32, tag="one_one")
            nc.vector.memset(one_one[:], 1.0)

            # resident input tiles
            emb_t, k_t, v_t = [], [], []
            for d in range(ND):
                sl = slice(d * 128, (d + 1) * 128)
                for lst, dram, nm, dt_ in (
                    (emb_t, embT, "e", MDT),
                    (k_t, kT_in, "k", MDT),
                    (v_t, vT_in, "v", F32),
                ):
                    t = iop.tile([128, S], dt_, tag=f"{nm}{d}")
                    if dt_ == F32:
                        nc.sync.dma_start(t[:], dram[sl, :])
                    else:
                        dma_in(t[:], dram[sl, :])
                    lst.append(t)

            # weights: 4 x [128, 64] chunks + [1, 64] bias row each
            def load_w(dram, nm, wdt=None):
                wdt = MDT if wdt is None else wdt
                chunks = []
                for d in range(ND):
                    t = wp.tile([128, dram.shape[1]], wdt, tag=f"{nm}{d}")
                    if wdt == F32:
                        nc.sync.dma_start(t[:], dram[d * 128 : (d + 1) * 128, :])
                    else:
                        dma_in(t[:], dram[d * 128 : (d + 1) * 128, :])
                    chunks.append(t)
                b = wp.tile([1, dram.shape[1]], wdt, tag=f"{nm}b")
                if wdt == F32:
                    nc.sync.dma_start(b[:], dram[D_MODEL : D_MODEL + 1, :])
                else:
                    dma_in(b[:], dram[D_MODEL : D_MODEL + 1, :])
                return chunks, b

            wq_t, wq_b = load_w(wq_in, "wq")
            wk_t, wk_b = load_w(wk_in, "wk")
            wv_t, wv_b = load_w(wv_in, "wv", wdt=F32)
            wo_sb = wp.tile([DK, D_MODEL], MDT, tag="wo")
            dma_in(wo_sb[:], wo_in[:, :])

            # compute body, repeated `reps` times
            for _rep in range(reps):
              # qT / kT projections: [64, 2048]
              qT = qkp.tile([DK, S], MDT, tag="qT")
              kT = qkp.tile([DK, S], MDT, tag="kT")
              for dst, wt, wb, src in ((qT, wq_t, wq_b, emb_t), (kT, wk_t, wk_b, k_t)):
                  for sc in range(NSC):
                      ssl = slice(sc * 512, (sc + 1) * 512)
                      ps = pacc.tile([DK, 512], F32, tag="acc")
                      for d in range(ND):
                          nc.tensor.matmul(
                              ps[:], wt[d][:], src[d][:, ssl],
                              start=(d == 0), stop=False,
                          )
                      nc.tensor.matmul(
                          ps[:], wb[:], ones_r[:, ssl], start=False, stop=True
                      )
                      nc.vector.tensor_copy(dst[:, ssl], ps[:])

              # v tiles: [128, 65] per key tile (col 64 = ones)
              v_sb = []
              for t in range(NT):
                  tsl = slice(t * 128, (t + 1) * 128)
                  pv = pacc.tile([128, DK + 1], F32, tag="acc")
                  for d in range(ND):
                      nc.tensor.matmul(
                          pv[:], v_t[d][:, tsl], wv_t[d][:],
                          start=(d == 0), stop=False,
                      )
                  nc.tensor.matmul(
                      pv[:], ones_row[:, tsl], wv_b[:], start=False, stop=True
                  )
                  vt = vp.tile([128, DK + 1], MDT, tag=f"v{t}")
                  nc.vector.tensor_copy(vt[:], pv[:])
                  v_sb.append(vt)

              # main attention loop: two query chunks as interleaved
              # independent streams, so each engine always has the other
              # stream's work to hide PE<->ACT semaphore latency and keep
              # the PE dense (HAM stays warm).
              for scp in range(NSC // 2):
                  streams = []
                  for sc in (2 * scp, 2 * scp + 1):
                      ssl = slice(sc * 512, (sc + 1) * 512)
                      po_t = pop.tile([DK + 1, 512], F32, tag="o")
                      streams.append((sc, ssl, po_t))
                  for t in range(NT):
                      tsl = slice(t * 128, (t + 1) * 128)
                      exs = []
                      for sc, ssl, po_t in streams:
                          ps_s = pacc.tile([128, 512], F32, tag="acc")
                          nc.tensor.matmul(
                              ps_s[:], kT[:, tsl], qT[:, ssl],
                              start=True, stop=True,
                          )
                          ex = ep.tile([128, 512], MDT, tag="ex")
                          nc.scalar.activation(
                              ex[:], ps_s[:], mybir.ActivationFunctionType.Exp,
                              scale=0.125,
                          )
                          exs.append(ex)
                      for (sc, ssl, po_t), ex in zip(streams, exs):
                          nc.tensor.matmul(
                              po_t[:], v_sb[t][:], ex[:],
                              start=(t == 0), stop=(t == NT - 1),
                              skip_group_check=True,
                          )
                  for sc, ssl, po_t in streams:
                      rec = smp.tile([1, 512], F32, tag="rec")
                      nc.vector.reciprocal(rec[:], po_t[DK : DK + 1, :])
                      numT = nump.tile([DK, 512], MDT, tag="numT")
                      nc.vector.tensor_copy(numT[:], po_t[0:DK, :])
                      for j in range(4):
                          jsl = slice(j * 128, (j + 1) * 128)
                          prj = prp.tile([128, 1], F32, tag="r")
                          nc.tensor.matmul(
                              prj[:], rec[0:1, jsl], one_one[:],
                              start=True, stop=True,
                          )
                          rP = smp.tile([128, 1], F32, tag="rP")
                          nc.vector.tensor_copy(rP[:], prj[:])
                          ppj = ppp.tile([128, 512], F32, tag="p")
                          nc.tensor.matmul(
                              ppj[:], numT[:, jsl], wo_sb[:],
                              start=True, stop=True,
                          )
                          ob = outp.tile([128, 512], F32, tag="ob")
                          nc.vector.tensor_scalar_mul(ob[:], ppj[:], rP[:])
                          nc.sync.dma_start(
                              out[sc * 512 + j * 128 : sc * 512 + (j + 1) * 128, :],
                              ob[:],
                          )

    _split_multi_waits(nc)
    return nc


_NC = None


def _get_nc():
    global _NC
    if _NC is None:
        _NC = build_program()
    return _NC


def make_in_maps(inputs):
    """Host-side shard: transpose the shared activations once, slice
    per-head weights, fold biases in as an extra weight row."""
    emb = np.asarray(inputs["embeddings"], np.float32)
    K = np.asarray(inputs["K"], np.float32)
    V = np.asarray(inputs["V"], np.float32)
    Wq = np.asarray(inputs["Wq"], np.float32)
    bq = np.asarray(inputs["bq"], np.float32)
    Wk = np.asarray(inputs["Wk"], np.float32)
    bk = np.asarray(inputs["bk"], np.float32)
    Wv = np.asarray(inputs["Wv"], np.float32)
    bv = np.asarray(inputs["bv"], np.float32)
    Wo = np.asarray(inputs["Wo"], np.float32)

    embT = np.ascontiguousarray(emb.T)
    kT = np.ascontiguousarray(K.T)
    vT = np.ascontiguousarray(V.T)

    in_maps = []
    for h in range(N_CORES):
        in_maps.append(
            {
                "embT": embT,
                "kT_in": kT,
                "vT_in": vT,
                "wq": np.ascontiguousarray(
                    np.concatenate([Wq[h], bq[h][None, :]], axis=0)
                ),
                "wk": np.ascontiguousarray(
                    np.concatenate([Wk[h], bk[h][None, :]], axis=0)
                ),
                "wv": np.ascontiguousarray(
                    np.concatenate(
                        [
                            np.concatenate([Wv[h], bv[h][None, :]], axis=0),
                            np.concatenate(
                                [np.zeros((512, 1), np.float32),
                                 np.ones((1, 1), np.float32)], axis=0
                            ),
                        ],
                        axis=1,
                    )
                ),
                "wo": np.ascontiguousarray(Wo[h * DK : (h + 1) * DK, :]),
            }
        )
    return in_maps


def kernel(**inputs):
    nc = _get_nc()
    in_maps = make_in_maps(inputs)
    res = run_bass_kernel_spmd(nc, in_maps, core_ids=list(range(N_CORES)))
    bo = np.asarray(inputs["bo"], np.float32)
    acc = res.results[0]["out"].copy()
    for c in range(1, N_CORES):
        acc += res.results[c]["out"]
    return (acc + bo[None, :]).astype(np.float32)

